# revision 1
# baseline (speedup 1.0000x reference)
# CenterNet decode (pseudo-NMS + per-class topk + global topk + gathers)
# Trainium2 Bass/Tile kernel, SPMD over 8 NeuronCores (4 images per core).
#
# Math: the reference's two-stage topk (per-class top-K, then global top-K
# over the C*K candidates, both via stable jax.lax.top_k) is exactly a single
# global top-K over the NMS'd heatmap flattened [C*H*W], ties broken by
# ascending flat index. Per (image, class)-plane we extract candidates as the
# top-8 raw values of each 8-row chunk (with 1-row halos), decide 3x3-NMS
# suppression exactly among each chunk's own candidates (a suppressor of a
# rank<=8 candidate always ranks <=8 in the same haloed chunk), then per-plane
# top-8 and a per-image 13-round max8/match_replace merge — all preserving the
# exact (value desc, flat-index asc) order, including duplicate values.
import numpy as np

B_FULL, C, H, W = 32, 80, 128, 128
HW = H * W
K = 100
N_CORES = 8
B_LOC = B_FULL // N_CORES          # 4 images per core
P_TOT = B_LOC * C                  # 320 planes per core
NCHUNK = 16                        # 8-row chunks per plane
CAND = NCHUNK * 8                  # 128 candidates per plane
PAD = 128                          # halo pad elems (one row) each side
FPADW = HW + 2 * PAD
MERGE_W = C * 8                    # 640 candidates per image
NROUND = 13                        # ceil(104/8) extraction rounds
NW = NROUND * 8                    # 104 extracted winners


def build(num_devices=N_CORES, repeat=1):
    import concourse.bacc as bacc
    import concourse.mybir as mybir
    import concourse.tile as tile
    from concourse.alu_op_type import AluOpType as Op

    f32 = mybir.dt.float32
    u32 = mybir.dt.uint32

    nc = bacc.Bacc("TRN2", target_bir_lowering=False, debug=False,
                   num_devices=num_devices)
    fmap = nc.declare_dram_parameter("fmap", [P_TOT, HW], f32, isOutput=False)
    wh = nc.declare_dram_parameter("wh", [B_LOC, 2 * HW], f32, isOutput=False)
    reg = nc.declare_dram_parameter("reg", [B_LOC, 2 * HW], f32, isOutput=False)
    o_bb = nc.declare_dram_parameter("bboxes", [B_LOC, 4 * K], f32, isOutput=True)
    o_sc = nc.declare_dram_parameter("scores", [B_LOC, K], f32, isOutput=True)
    o_cl = nc.declare_dram_parameter("clses", [B_LOC, K], f32, isOutput=True)

    with tile.TileContext(nc) as tc:
        with tc.tile_pool(name="const", bufs=1) as cpool, \
             tc.tile_pool(name="big", bufs=2) as bigp, \
             tc.tile_pool(name="mid", bufs=2) as midp, \
             tc.tile_pool(name="sm", bufs=2) as smp, \
             tc.tile_pool(name="psum", bufs=2, space="PSUM") as pp:

            # ---- constants ----
            iota_row = cpool.tile([128, MERGE_W], f32)   # 0..639 each row
            nc.gpsimd.iota(iota_row[:], pattern=[[1, MERGE_W]], base=0,
                           channel_multiplier=0, allow_small_or_imprecise_dtypes=True)
            iota_col = cpool.tile([128, 1], f32)
            nc.gpsimd.iota(iota_col[:], pattern=[[0, 1]], base=0,
                           channel_multiplier=1, allow_small_or_imprecise_dtypes=True)
            ident = cpool.tile([128, 128], f32)
            nc.vector.tensor_scalar(out=ident[:], in0=iota_row[:, 0:128],
                                    scalar1=iota_col[:], scalar2=None, op0=Op.is_equal)
            chunkbase = cpool.tile([128, CAND], f32)     # (j,r) -> 1024j-128
            nc.gpsimd.iota(chunkbase[:].rearrange("p (j r) -> p j r", j=NCHUNK),
                           pattern=[[1024, NCHUNK], [0, 8]], base=-PAD,
                           channel_multiplier=0, allow_small_or_imprecise_dtypes=True)
            ones1 = cpool.tile([1, 128], f32)
            nc.vector.memset(ones1[:], 1.0)

            for _rep in range(repeat):
                # gathered per-plane top-8 (v | s), per-image rows
                mg = smp.tile([B_LOC, 16 * C], f32, tag="mg")
                # ================= per-tile candidate extraction ===========
                for t, P in enumerate([128, 128, 64]):
                    Pb = 128 * t
                    fpad = bigp.tile([128, FPADW], f32, tag="fpad")
                    nc.gpsimd.memset(fpad[:P, 0:PAD], -1.0)
                    nc.gpsimd.memset(fpad[:P, PAD + HW:FPADW], -1.0)
                    nc.sync.dma_start(out=fpad[:P, PAD:PAD + HW],
                                      in_=fmap.ap()[Pb:Pb + P, :])

                    cm_v = midp.tile([128, CAND], f32, tag="cm_v")
                    cm_i = midp.tile([128, CAND], u32, tag="cm_i")
                    for j in range(NCHUNK):
                        win = fpad[:P, 1024 * j:1024 * j + 1280]
                        nc.vector.max(cm_v[:P, 8 * j:8 * j + 8], win)
                        nc.vector.max_index(cm_i[:P, 8 * j:8 * j + 8],
                                            cm_v[:P, 8 * j:8 * j + 8], win)

                    cm_if = midp.tile([128, CAND], f32, tag="cm_if")
                    nc.vector.tensor_copy(cm_if[:P], cm_i[:P])
                    s_f = midp.tile([128, CAND], f32, tag="s_f")
                    nc.vector.tensor_tensor(out=s_f[:P], in0=cm_if[:P],
                                            in1=chunkbase[:P], op=Op.add)
                    # core mask: idx in [128, 1152)  <=>  |idx-639.5| <= 511.5
                    mc = midp.tile([128, CAND], f32, tag="mc")
                    nc.vector.tensor_scalar(out=mc[:P], in0=cm_if[:P], scalar1=639.5,
                                            scalar2=0.0, op0=Op.subtract, op1=Op.abs_max)
                    nc.vector.tensor_scalar(out=mc[:P], in0=mc[:P], scalar1=511.5,
                                            scalar2=None, op0=Op.is_le)
                    xs = midp.tile([128, CAND], f32, tag="xs")
                    nc.vector.tensor_scalar(out=xs[:P], in0=s_f[:P], scalar1=float(W),
                                            scalar2=None, op0=Op.mod)
                    ys = midp.tile([128, CAND], f32, tag="ys")
                    nc.vector.tensor_tensor(out=ys[:P], in0=s_f[:P], in1=xs[:P],
                                            op=Op.subtract)          # = 128*ys
                    # ---- suppression: all ordered pairs (r1 suppressor, r2) ----
                    def Asp(x):
                        return x[:P].rearrange("p (j r) -> p j r", j=NCHUNK) \
                                    .unsqueeze(3).broadcast_to((P, NCHUNK, 8, 8))
                    def Bsp(x):
                        return x[:P].rearrange("p (j r) -> p j r", j=NCHUNK) \
                                    .unsqueeze(2).broadcast_to((P, NCHUNK, 8, 8))
                    wide = lambda tag: midp.tile([128, NCHUNK * 64], f32, tag=tag)
                    d3 = lambda x: x[:P].rearrange("p (j a b) -> p j a b", j=NCHUNK, a=8)
                    ex = wide("ex"); ey = wide("ey"); adj = wide("adj")
                    nc.vector.tensor_tensor(out=d3(ex), in0=Asp(xs), in1=Bsp(xs),
                                            op=Op.subtract)
                    nc.vector.tensor_scalar(out=ex[:P], in0=ex[:P], scalar1=0.0,
                                            scalar2=1.5, op0=Op.abs_max, op1=Op.is_le)
                    nc.vector.tensor_tensor(out=d3(ey), in0=Asp(ys), in1=Bsp(ys),
                                            op=Op.subtract)           # 128*dy
                    nc.vector.tensor_scalar(out=ey[:P], in0=ey[:P], scalar1=0.0,
                                            scalar2=192.0, op0=Op.abs_max, op1=Op.is_le)
                    nc.vector.tensor_tensor(out=adj[:P], in0=ex[:P], in1=ey[:P],
                                            op=Op.mult)
                    nc.vector.tensor_tensor(out=d3(ex), in0=Asp(cm_v), in1=Bsp(cm_v),
                                            op=Op.is_gt)              # reuse ex as gt
                    nc.vector.tensor_tensor(out=adj[:P], in0=adj[:P], in1=ex[:P],
                                            op=Op.mult)
                    supp = midp.tile([128, CAND], f32, tag="supp")
                    nc.vector.tensor_reduce(
                        supp[:P].rearrange("p (j b) -> p j b", j=NCHUNK),
                        adj[:P].rearrange("p (j a b) -> p j b a", j=NCHUNK, a=8),
                        axis=mybir.AxisListType.X, op=Op.max)
                    # alive = (supp == 0) * core_mask ; cv = cm_v * alive
                    nc.vector.tensor_scalar(out=supp[:P], in0=supp[:P], scalar1=0.0,
                                            scalar2=None, op0=Op.is_equal)
                    nc.vector.tensor_tensor(out=supp[:P], in0=supp[:P], in1=mc[:P],
                                            op=Op.mult)
                    cv = midp.tile([128, CAND], f32, tag="cv")
                    nc.vector.tensor_tensor(out=cv[:P], in0=cm_v[:P], in1=supp[:P],
                                            op=Op.mult)
                    # ---- per-plane top-8 (+ gather spatial idx per slot) ----
                    t8 = midp.tile([128, 16], f32, tag="t8")
                    nc.vector.max(t8[:P, 0:8], cv[:P, :])
                    t8i = midp.tile([128, 8], u32, tag="t8i")
                    nc.vector.max_index(t8i[:P], t8[:P, 0:8], cv[:P, :])
                    t8if = midp.tile([128, 8], f32, tag="t8if")
                    nc.vector.tensor_copy(t8if[:P], t8i[:P])
                    ohb = midp.tile([128, CAND], f32, tag="ohb")
                    dmb = midp.tile([128, CAND], f32, tag="dmb")
                    for r in range(8):
                        nc.vector.tensor_scalar(out=ohb[:P], in0=iota_row[:P, 0:CAND],
                                                scalar1=t8if[:P, r:r + 1], scalar2=None,
                                                op0=Op.is_equal)
                        nc.vector.tensor_tensor_reduce(
                            out=dmb[:P], in0=ohb[:P], in1=s_f[:P], scale=1.0,
                            scalar=0.0, op0=Op.mult, op1=Op.add,
                            accum_out=t8[:P, 8 + r:9 + r])
                    # partition-fold into per-image rows of mg (SBUF->SBUF DMA)
                    # and spatial idx rows into msall (partition 0, for matmul rhs)
                    p0 = 0
                    while p0 < P:
                        plane = Pb + p0
                        b = plane // C
                        c0 = plane - b * C
                        n = min(P - p0, C - c0)
                        nc.sync.dma_start(
                            out=mg[b:b + 1, 16 * c0:16 * (c0 + n)],
                            in_=t8[p0:p0 + n, :])
                        nc.sync.dma_start(
                            out=msall[0:1, MERGE_W * b + 8 * c0:MERGE_W * b + 8 * (c0 + n)],
                            in_=t8[p0:p0 + n, 8:16])
                        p0 += n

                # ================= per-image merge =========================
                mv = [smp.tile([B_LOC, MERGE_W], f32, tag=f"mv{i}") for i in range(2)]
                msp = smp.tile([B_LOC, MERGE_W], f32, tag="msp")
                mg3 = mg[:].rearrange("b (c f) -> b c f", c=C)
                nc.vector.tensor_copy(mv[0][:].rearrange("b (c r) -> b c r", c=C),
                                      mg3[:, :, 0:8])
                nc.vector.tensor_copy(msp[:].rearrange("b (c r) -> b c r", c=C),
                                      mg3[:, :, 8:16])
                wv = smp.tile([B_LOC, NW], f32, tag="wv")
                wsl = smp.tile([B_LOC, NW], u32, tag="wsl")
                for k in range(NROUND):
                    src, dst = mv[k % 2], mv[(k + 1) % 2]
                    nc.vector.max(wv[:, 8 * k:8 * k + 8], src[:])
                    nc.vector.max_index(wsl[:, 8 * k:8 * k + 8],
                                        wv[:, 8 * k:8 * k + 8], src[:])
                    if k < NROUND - 1:
                        nc.vector.match_replace(dst[:], in_to_replace=wv[:, 8 * k:8 * k + 8],
                                                in_values=src[:], imm_value=-1.0)
                wslf = smp.tile([B_LOC, NW], f32, tag="wslf")
                nc.vector.tensor_copy(wslf[:], wsl[:])

                # ---- transpose winners to [NW, B_LOC] ----
                def transpose_to(dst_tile, src_ap, p_in, f_in, tag):
                    ps = pp.tile([f_in, p_in], f32, tag=tag)
                    nc.tensor.transpose(ps[:], src_ap, ident[0:p_in, 0:p_in])
                    nc.vector.tensor_copy(dst_tile, ps[:])
                    return dst_tile
                wslT = smp.tile([NW, B_LOC], f32, tag="wslT")
                transpose_to(wslT[:], wslf[:], B_LOC, NW, "psT")

                # cls = floor(slot/8); r = slot mod 8  (on [NW, B_LOC])
                rT = smp.tile([NW, B_LOC], f32, tag="rT")
                nc.vector.tensor_scalar(out=rT[:], in0=wslT[:], scalar1=8.0,
                                        scalar2=None, op0=Op.mod)
                clsT = smp.tile([NW, B_LOC], f32, tag="clsT")
                nc.vector.tensor_tensor(out=clsT[:], in0=wslT[:], in1=rT[:],
                                        op=Op.subtract)
                nc.vector.tensor_scalar(out=clsT[:], in0=clsT[:], scalar1=0.125,
                                        scalar2=None, op0=Op.mult)

                # ---- resolve spatial index s per winner ----
                sT = smp.tile([NW, B_LOC], f32, tag="sT")
                ohw = smp.tile([NW, MERGE_W], f32, tag="ohw")
                dmw = smp.tile([NW, MERGE_W], f32, tag="dmw")
                acc2 = smp.tile([NW, 2], f32, tag="acc2")
                for b in range(B_LOC):
                    ps1 = pp.tile([NW, 512], f32, tag="ps1")
                    ps2 = pp.tile([NW, MERGE_W - 512], f32, tag="ps2")
                    nc.tensor.matmul(ps1[:], lhsT=ones1[:, 0:NW],
                                     rhs=msp[b:b + 1, 0:512], start=True, stop=True)
                    nc.tensor.matmul(ps2[:], lhsT=ones1[:, 0:NW],
                                     rhs=msp[b:b + 1, 512:MERGE_W], start=True, stop=True)
                    nc.vector.tensor_scalar(out=ohw[:], in0=iota_row[0:NW, :],
                                            scalar1=wslT[:, b:b + 1], scalar2=None,
                                            op0=Op.is_equal)
                    nc.vector.tensor_tensor_reduce(
                        out=dmw[:, 0:512], in0=ohw[:, 0:512], in1=ps1[:], scale=1.0,
                        scalar=0.0, op0=Op.mult, op1=Op.add, accum_out=acc2[:, 0:1])
                    nc.vector.tensor_tensor_reduce(
                        out=dmw[:, 512:MERGE_W], in0=ohw[:, 512:MERGE_W], in1=ps2[:],
                        scale=1.0, scalar=0.0, op0=Op.mult, op1=Op.add,
                        accum_out=acc2[:, 1:2])
                    nc.vector.tensor_tensor(out=sT[:, b:b + 1], in0=acc2[:, 0:1],
                                            in1=acc2[:, 1:2], op=Op.add)

                xsT = smp.tile([NW, B_LOC], f32, tag="xsT")
                nc.vector.tensor_scalar(out=xsT[:], in0=sT[:], scalar1=float(W),
                                        scalar2=None, op0=Op.mod)
                ysT = smp.tile([NW, B_LOC], f32, tag="ysT")
                nc.vector.tensor_tensor(out=ysT[:], in0=sT[:], in1=xsT[:],
                                        op=Op.subtract)
                nc.vector.tensor_scalar(out=ysT[:], in0=ysT[:], scalar1=1.0 / W,
                                        scalar2=None, op0=Op.mult)

                # ---- gather wh/reg at (ys, xs) via one-hot matmuls ----
                gath = smp.tile([NW, 4 * B_LOC], f32, tag="gath")  # cols 4b+{w,h,rx,ry}
                ohy = smp.tile([NW, 128], f32, tag="ohy")
                ohx = smp.tile([NW, 128], f32, tag="ohx")
                ohyT = smp.tile([128, NW], f32, tag="ohyT")
                dm2 = smp.tile([NW, 128], f32, tag="dm2")
                for b in range(B_LOC):
                    whreg = midp.tile([128, 512], f32, tag="whreg")
                    nc.sync.dma_start(out=whreg[:, 0:128],
                                      in_=wh.ap()[b, 0:HW].rearrange("(y x) -> y x", y=H))
                    nc.sync.dma_start(out=whreg[:, 128:256],
                                      in_=wh.ap()[b, HW:2 * HW].rearrange("(y x) -> y x", y=H))
                    nc.sync.dma_start(out=whreg[:, 256:384],
                                      in_=reg.ap()[b, 0:HW].rearrange("(y x) -> y x", y=H))
                    nc.sync.dma_start(out=whreg[:, 384:512],
                                      in_=reg.ap()[b, HW:2 * HW].rearrange("(y x) -> y x", y=H))
                    nc.vector.tensor_scalar(out=ohy[:], in0=iota_row[0:NW, 0:128],
                                            scalar1=ysT[:, b:b + 1], scalar2=None,
                                            op0=Op.is_equal)
                    psy = pp.tile([128, NW], f32, tag="psy")
                    nc.tensor.transpose(psy[:], ohy[:], ident[0:NW, 0:NW])
                    nc.vector.tensor_copy(ohyT[:], psy[:])
                    psm = pp.tile([NW, 512], f32, tag="psm")
                    nc.tensor.matmul(psm[:], lhsT=ohyT[:, 0:NW], rhs=whreg[:],
                                     start=True, stop=True)
                    nc.vector.tensor_scalar(out=ohx[:], in0=iota_row[0:NW, 0:128],
                                            scalar1=xsT[:, b:b + 1], scalar2=None,
                                            op0=Op.is_equal)
                    for ch in range(4):
                        nc.vector.tensor_tensor_reduce(
                            out=dm2[:], in0=psm[:, 128 * ch:128 * ch + 128], in1=ohx[:],
                            scale=1.0, scalar=0.0, op0=Op.mult, op1=Op.add,
                            accum_out=gath[:, 4 * b + ch:4 * b + ch + 1])

                # ---- decode boxes ----
                wsl4 = lambda c0: gath[:].rearrange("p (b c) -> p b c", b=B_LOC)[:, :, c0]
                xc = smp.tile([NW, B_LOC], f32, tag="xc")
                yc = smp.tile([NW, B_LOC], f32, tag="yc")
                hw2 = smp.tile([NW, B_LOC], f32, tag="hw2")
                hh2 = smp.tile([NW, B_LOC], f32, tag="hh2")
                nc.vector.tensor_tensor(out=xc[:], in0=xsT[:], in1=wsl4(2), op=Op.add)
                nc.vector.tensor_tensor(out=yc[:], in0=ysT[:], in1=wsl4(3), op=Op.add)
                nc.vector.tensor_scalar(out=hw2[:], in0=wsl4(0), scalar1=0.5,
                                        scalar2=None, op0=Op.mult)
                nc.vector.tensor_scalar(out=hh2[:], in0=wsl4(1), scalar1=0.5,
                                        scalar2=None, op0=Op.mult)
                bbp = smp.tile([NW, 4 * B_LOC], f32, tag="bbp")  # cols 4b+comp
                bb3 = bbp[:].rearrange("p (b c) -> p b c", b=B_LOC)
                nc.vector.tensor_tensor(out=bb3[:, :, 0], in0=xc[:], in1=hw2[:], op=Op.subtract)
                nc.vector.tensor_tensor(out=bb3[:, :, 1], in0=yc[:], in1=hh2[:], op=Op.subtract)
                nc.vector.tensor_tensor(out=bb3[:, :, 2], in0=xc[:], in1=hw2[:], op=Op.add)
                nc.vector.tensor_tensor(out=bb3[:, :, 3], in0=yc[:], in1=hh2[:], op=Op.add)

                # ---- pack + write outputs ----
                bbT = smp.tile([4 * B_LOC, NW], f32, tag="bbT")
                transpose_to(bbT[:], bbp[:], NW, 4 * B_LOC, "psbb")
                bbrow = smp.tile([B_LOC, 4 * K], f32, tag="bbrow")
                for comp in range(4):
                    nc.vector.tensor_copy(
                        bbrow[:].rearrange("b (k c) -> b k c", k=K)[:, :, comp],
                        bbT[comp:4 * B_LOC:4, 0:K])
                nc.sync.dma_start(out=o_bb.ap(), in_=bbrow[:])
                nc.sync.dma_start(out=o_sc.ap(), in_=wv[:, 0:K])
                clT2 = smp.tile([B_LOC, NW], f32, tag="clT2")
                transpose_to(clT2[:], clsT[:], NW, B_LOC, "pscl")
                nc.sync.dma_start(out=o_cl.ap(), in_=clT2[:, 0:K])

    nc.compile()
    return nc


_NC_CACHE = {}


def _get_nc():
    if "nc" not in _NC_CACHE:
        _NC_CACHE["nc"] = build()
    return _NC_CACHE["nc"]


def kernel(fmap, wh, reg, K):
    from concourse.bass_utils import run_bass_kernel_spmd
    fmap = np.ascontiguousarray(np.asarray(fmap, dtype=np.float32))
    wh = np.ascontiguousarray(np.asarray(wh, dtype=np.float32))
    reg = np.ascontiguousarray(np.asarray(reg, dtype=np.float32))
    nc = _get_nc()
    in_maps = []
    for c in range(N_CORES):
        sl = slice(c * B_LOC, (c + 1) * B_LOC)
        in_maps.append({
            "fmap": fmap[sl].reshape(P_TOT, HW),
            "wh": wh[sl].reshape(B_LOC, 2 * HW),
            "reg": reg[sl].reshape(B_LOC, 2 * HW),
        })
    res = run_bass_kernel_spmd(nc, in_maps, core_ids=list(range(N_CORES)))
    bb = np.concatenate([r["bboxes"].reshape(B_LOC, 100, 4) for r in res.results])
    sc = np.concatenate([r["scores"].reshape(B_LOC, 100, 1) for r in res.results])
    cl = np.concatenate([r["clses"].reshape(B_LOC, 100, 1) for r in res.results])
    return bb, sc, cl
